# revision 6
# baseline (speedup 1.0000x reference)
"""BiLSTM-CRF loss kernel for Trainium2 (8 NeuronCores, data-parallel over batch).

Self-contained: hardcodes shapes B=128, T=512, V=50000, NT=24, E=128, H=256.
Each core processes 16 examples end-to-end (embedding gather, BiLSTM,
emissions, CRF forward logZ, gold path score); host only reorders inputs into
DMA-friendly layouts, builds one-hot index selectors, and averages the 128
per-example (logZ - gold) values.
"""

import sys

for _p in ("/opt/trn_rl_repo",):
    if _p not in sys.path:
        sys.path.insert(0, _p)

import numpy as np
import ml_dtypes

import concourse.bass as bass
import concourse.bacc as bacc
import concourse.tile as tile
from concourse import mybir
from concourse.bass import IndirectOffsetOnAxis
from concourse.masks import make_identity

F32 = mybir.dt.float32
BF16 = mybir.dt.bfloat16
I32 = mybir.dt.int32
U8 = mybir.dt.uint8
AX = mybir.AxisListType
OP = mybir.AluOpType
ACTF = mybir.ActivationFunctionType


def full_cfg():
    return dict(T=512, Bl=16, V=50000, NT=24, E=128, Hd=128, Tc=32, UT=256,
                EC=512, TG=32)


def shift_steps(cfg):
    # steps at which the CRF running score is re-based (every 2nd step)
    return [t for t in range(2, cfg["T"], 2)]


def build_body(tc, outs, ins, cfg):
    """Emit the whole per-core program inside an open TileContext.

    outs/ins: dicts name -> bass.AP (DRAM).
    """
    nc = tc.nc
    T, Bl, NT, Hd = cfg["T"], cfg["Bl"], cfg["NT"], cfg["Hd"]
    Tc, UT, EC, TG = cfg["Tc"], cfg["UT"], cfg["EC"], cfg["TG"]
    R = T * Bl                  # total (t, b) rows
    M = R // 128                # 128-row tiles
    NCH = T // Tc               # lstm chunks
    RTC = Tc * Bl // 128        # row-tiles per chunk
    G4 = 4 * Hd
    shifts = shift_steps(cfg)
    shift_of = {t: i for i, t in enumerate(shifts)}

    import contextlib
    ctx = contextlib.ExitStack()
    with ctx:
        const = ctx.enter_context(tc.tile_pool(name="const", bufs=1))
        big = ctx.enter_context(tc.tile_pool(name="big", bufs=1))
        work = ctx.enter_context(tc.tile_pool(name="work", bufs=3))

        # ---------------- constants ----------------
        ident = const.tile([128, 128], F32)
        make_identity(nc, ident[:])

        idx_sb = const.tile([128, M], I32)
        nc.sync.dma_start(out=idx_sb[:], in_=ins["idx"][:])

        wih_sb = const.tile([128, 2, G4], BF16)
        nc.sync.dma_start(out=wih_sb[:], in_=ins["wih"][:])
        whh_sb = const.tile([128, 2, G4], BF16)
        nc.sync.dma_start(out=whh_sb[:], in_=ins["whh"][:])
        wout_sb = const.tile([128, 2, NT], BF16)
        nc.sync.dma_start(out=wout_sb[:], in_=ins["wout"][:])

        biasin = const.tile([128, 2, 2, 4], F32)
        nc.sync.dma_start(out=biasin[:], in_=ins["biasin"][:])
        bias_sb = const.tile([128, 2, 4], F32)
        nc.vector.tensor_add(bias_sb[:], biasin[:, :, 0, :], biasin[:, :, 1, :])

        trans_sb = const.tile([NT, NT], F32)
        nc.sync.dma_start(out=trans_sb[:], in_=ins["trans"][:])
        E_sb = const.tile([NT, NT], F32)
        nc.scalar.activation(E_sb[:], trans_sb[:], ACTF.Exp)

        bout_sb = const.tile([NT, 1], F32)
        nc.sync.dma_start(out=bout_sb[:], in_=ins["bout"][:])
        startv = const.tile([NT, 1], F32)
        nc.sync.dma_start(out=startv[:], in_=ins["startv"][:])
        endv = const.tile([NT, 1], F32)
        nc.sync.dma_start(out=endv[:], in_=ins["endv"][:])
        selstart = const.tile([NT, Bl], F32)
        nc.sync.dma_start(out=selstart[:], in_=ins["selstart"][:])
        selend = const.tile([NT, Bl], F32)
        nc.sync.dma_start(out=selend[:], in_=ins["selend"][:])

        nsh = len(shifts)
        maskA = const.tile([1, nsh * Bl], F32)
        nc.sync.dma_start(out=maskA[:], in_=ins["maskA"][:])
        maskc = const.tile([NT, (T - UT) * Bl], U8)
        nc.sync.dma_start(out=maskc[:], in_=ins["maskc"][:])

        ones24 = const.tile([NT, 1], F32)
        nc.vector.memset(ones24[:], 1.0)
        ones1 = const.tile([1, NT], F32)
        nc.vector.memset(ones1[:], 1.0)

        # ---------------- phase A: gather + transpose embeddings ----------------
        xeT = big.tile([128, R], BF16)
        with tc.tile_pool(name="psA", bufs=2, space="PSUM") as psA, \
             tc.tile_pool(name="gath", bufs=3) as gath:
            for m in range(M):
                xe_raw = gath.tile([128, 128], F32, tag="xe_raw")
                nc.gpsimd.indirect_dma_start(
                    out=xe_raw[:], out_offset=None,
                    in_=ins["emb"][:],
                    in_offset=IndirectOffsetOnAxis(ap=idx_sb[:, m:m + 1], axis=0),
                )
                xe_ps = psA.tile([128, 128], F32, tag="xe_ps")
                nc.tensor.transpose(xe_ps[:], xe_raw[:], ident[:])
                nc.vector.tensor_copy(xeT[:, m * 128:(m + 1) * 128], xe_ps[:])

            # ---------------- phase B: BiLSTM ----------------
            h_f = big.tile([128, R], BF16)
            h_b = big.tile([128, R], BF16)
            zero_h = const.tile([128, 2 * Bl], BF16)
            nc.vector.memset(zero_h[:], 0.0)
            c_comb = big.tile([128, 2 * Bl], F32)
            nc.vector.memset(c_comb[:], 0.0)

            with tc.tile_pool(name="xp", bufs=2) as xp_pool, \
                 tc.tile_pool(name="lwork", bufs=3) as lwork:

                def produce_chunk(d, ch):
                    xp_t = xp_pool.tile([128, 4, Tc * Bl], F32, tag=f"xp{d}")
                    for rt in range(RTC):
                        row0 = ch * Tc * Bl + rt * 128
                        ps = psA.tile([128, G4], F32, tag="xp_ps")
                        for k in range(4):
                            nc.tensor.matmul(
                                ps[:, k * 128:(k + 1) * 128],
                                lhsT=wih_sb[:, d, k * 128:(k + 1) * 128],
                                rhs=xeT[:, row0:row0 + 128],
                                start=True, stop=True)
                        ps3 = ps[:].rearrange("p (k r) -> p k r", k=4)
                        nc.vector.tensor_add(
                            xp_t[:, :, rt * 128:(rt + 1) * 128],
                            ps3,
                            bias_sb[:, d, :].unsqueeze(2).to_broadcast(
                                (128, 4, 128)))
                    return xp_t

                for ci in range(NCH):
                    chf, chb = ci, NCH - 1 - ci
                    xpf = produce_chunk(0, chf)
                    xpb = produce_chunk(1, chb)
                    for sl in range(Tc):
                        s = ci * Tc + sl
                        tf, tb = s, T - 1 - s
                        rhs_f = (h_f[:, (tf - 1) * Bl:tf * Bl] if tf > 0
                                 else zero_h[:, 0:Bl])
                        rhs_b = (h_b[:, (tb + 1) * Bl:(tb + 2) * Bl] if tb < T - 1
                                 else zero_h[:, Bl:2 * Bl])
                        gps = psA.tile([128, 4, 2 * Bl], F32, tag="gps")
                        for k in range(4):
                            nc.tensor.matmul(
                                gps[:, k, 0:Bl],
                                lhsT=whh_sb[:, 0, k * 128:(k + 1) * 128],
                                rhs=rhs_f, start=True, stop=True)
                            nc.tensor.matmul(
                                gps[:, k, Bl:2 * Bl],
                                lhsT=whh_sb[:, 1, k * 128:(k + 1) * 128],
                                rhs=rhs_b, start=True, stop=True)
                        g = lwork.tile([128, 4, 2 * Bl], F32, tag="g")
                        lf = sl * Bl
                        lb = (Tc - 1 - sl) * Bl
                        nc.vector.tensor_add(
                            g[:, :, 0:Bl], gps[:, :, 0:Bl],
                            xpf[:, :, lf:lf + Bl])
                        nc.vector.tensor_add(
                            g[:, :, Bl:2 * Bl], gps[:, :, Bl:2 * Bl],
                            xpb[:, :, lb:lb + Bl])
                        sg = lwork.tile([128, 4, 2 * Bl], F32, tag="sg")
                        nc.scalar.activation(sg[:, 0:3, :], g[:, 0:3, :],
                                             ACTF.Sigmoid)
                        nc.scalar.activation(sg[:, 3, :], g[:, 3, :], ACTF.Tanh)
                        m1 = lwork.tile([128, 2 * Bl], F32, tag="m1")
                        nc.vector.tensor_mul(m1[:], sg[:, 0, :], sg[:, 3, :])
                        m2 = lwork.tile([128, 2 * Bl], F32, tag="m2")
                        nc.vector.tensor_mul(m2[:], sg[:, 1, :], c_comb[:])
                        nc.vector.tensor_add(c_comb[:], m1[:], m2[:])
                        tcn = lwork.tile([128, 2 * Bl], F32, tag="tcn")
                        nc.scalar.activation(tcn[:], c_comb[:], ACTF.Tanh)
                        nc.vector.tensor_mul(
                            h_f[:, tf * Bl:(tf + 1) * Bl],
                            sg[:, 2, 0:Bl], tcn[:, 0:Bl])
                        nc.vector.tensor_mul(
                            h_b[:, tb * Bl:(tb + 1) * Bl],
                            sg[:, 2, Bl:2 * Bl], tcn[:, Bl:2 * Bl])

        # ---------------- phase C: emissions ----------------
        emT = big.tile([NT, R], F32)
        with tc.tile_pool(name="psB", bufs=2, space="PSUM") as psB:
            for ec in range(R // EC):
                ps = psB.tile([NT, EC], F32, tag="em_ps")
                nc.tensor.matmul(ps[:], lhsT=wout_sb[:, 0, :],
                                 rhs=h_f[:, ec * EC:(ec + 1) * EC],
                                 start=True, stop=False)
                nc.tensor.matmul(ps[:], lhsT=wout_sb[:, 1, :],
                                 rhs=h_b[:, ec * EC:(ec + 1) * EC],
                                 start=False, stop=True)
                nc.scalar.activation(emT[:, ec * EC:(ec + 1) * EC], ps[:],
                                     ACTF.Identity, bias=bout_sb[:])

        # ---------------- phase D: CRF forward + gold ----------------
        with tc.tile_pool(name="psC", bufs=2, space="PSUM") as psC, \
             tc.tile_pool(name="cwork", bufs=3) as cwork, \
             tc.tile_pool(name="gwork", bufs=2) as gwork:
            scoreT = big.tile([NT, Bl], F32)
            nc.scalar.activation(scoreT[:], emT[:, 0:Bl], ACTF.Identity,
                                 bias=startv[:])
            offs_hist = big.tile([1, max(nsh, 1) * Bl], F32)

            for t in range(1, T):
                pexp = cwork.tile([NT, Bl], F32, tag="pexp")
                if t in shift_of:
                    si = shift_of[t]
                    nc.scalar.copy(offs_hist[0:1, si * Bl:(si + 1) * Bl],
                                   scoreT[0:1, :])
                    ob = psC.tile([NT, Bl], F32, tag="ob")
                    nc.tensor.matmul(ob[:], lhsT=ones1[:],
                                     rhs=scoreT[0:1, :], start=True, stop=True)
                    sub = cwork.tile([NT, Bl], F32, tag="sub")
                    nc.vector.tensor_tensor(sub[:], scoreT[:], ob[:],
                                            op=OP.subtract)
                    nc.scalar.activation(pexp[:], sub[:], ACTF.Exp)
                else:
                    nc.scalar.activation(pexp[:], scoreT[:], ACTF.Exp)
                Pp = psC.tile([NT, Bl], F32, tag="Pp")
                nc.tensor.matmul(Pp[:], lhsT=E_sb[:], rhs=pexp[:],
                                 start=True, stop=True)
                lnp = cwork.tile([NT, Bl], F32, tag="lnp")
                nc.scalar.activation(lnp[:], Pp[:], ACTF.Ln)
                if t < UT:
                    nc.vector.tensor_add(scoreT[:], lnp[:],
                                         emT[:, t * Bl:(t + 1) * Bl])
                else:
                    ns = cwork.tile([NT, Bl], F32, tag="ns")
                    nc.vector.tensor_add(ns[:], lnp[:],
                                         emT[:, t * Bl:(t + 1) * Bl])
                    nc.vector.copy_predicated(
                        scoreT[:], maskc[:, (t - UT) * Bl:(t - UT + 1) * Bl],
                        ns[:])

            # logZ' = LSE_i(score'_i + end_i) per example
            sce = cwork.tile([NT, Bl], F32, tag="sce")
            nc.scalar.activation(sce[:], scoreT[:], ACTF.Identity, bias=endv[:])
            tp = psC.tile([Bl, NT], F32, tag="tiny")
            nc.tensor.transpose(tp[:], sce[:], ident[0:NT, 0:NT])
            sc2 = cwork.tile([Bl, NT], F32, tag="sc2")
            nc.vector.tensor_copy(sc2[:], tp[:])
            nmx = cwork.tile([Bl, 1], F32, tag="nmx")
            nc.vector.tensor_reduce(nmx[:], sc2[:], axis=AX.X, op=OP.max,
                                    negate=True)
            pe2 = cwork.tile([Bl, NT], F32, tag="pe2")
            se = cwork.tile([Bl, 1], F32, tag="se")
            nc.scalar.activation(pe2[:], sc2[:], ACTF.Exp, bias=nmx[:],
                                 accum_out=se[:])
            lse = cwork.tile([Bl, 1], F32, tag="lse")
            nc.scalar.activation(lse[:], se[:], ACTF.Ln)
            logZp = cwork.tile([Bl, 1], F32, tag="logZp")
            nc.vector.tensor_tensor(logZp[:], lse[:], nmx[:], op=OP.subtract)

            # A_b = sum over shift steps of offs * mask
            ohm = cwork.tile([1, max(nsh, 1) * Bl], F32, tag="ohm")
            nc.vector.tensor_mul(ohm[:], offs_hist[:], maskA[:])
            A_t = cwork.tile([1, Bl], F32, tag="A_t")
            ohm_v = ohm[:].rearrange("p (s b) -> p b s", b=Bl)
            nc.vector.tensor_reduce(A_t[:], ohm_v, axis=AX.X, op=OP.add)

            # ---- gold path score ----
            accE = gwork.tile([NT, Bl], F32, tag="accE")
            nc.vector.memset(accE[:], 0.0)
            accT = gwork.tile([NT, Bl], F32, tag="accT")
            nc.vector.memset(accT[:], 0.0)

            # emission term: sum_t w1hot[:, t, b] * emT[:, t, b]
            n_em_chunks = (T + TG - 1) // TG
            for gc in range(n_em_chunks):
                t0 = gc * TG
                nt_ = min(TG, T - t0)
                w1 = gwork.tile([NT, TG * Bl], F32, tag="w1")
                nc.sync.dma_start(
                    out=w1[:, 0:nt_ * Bl],
                    in_=ins["w1hot"][:, t0 * Bl:(t0 + nt_) * Bl])
                mm1 = gwork.tile([NT, TG * Bl], F32, tag="mm1")
                nc.vector.tensor_mul(mm1[:, 0:nt_ * Bl], w1[:, 0:nt_ * Bl],
                                     emT[:, t0 * Bl:(t0 + nt_) * Bl])
                mv = mm1[:, 0:nt_ * Bl].rearrange("p (t b) -> p b t", b=Bl)
                red = gwork.tile([NT, Bl], F32, tag="red")
                nc.vector.tensor_reduce(red[:], mv, axis=AX.X, op=OP.add)
                nc.vector.tensor_add(accE[:], accE[:], red[:])

            # transition term: sum_t sel2m[:,t,b] * (trans^T @ sel1)[:,t,b]
            n_tr_chunks = (T - 1 + TG - 1) // TG
            for gc in range(n_tr_chunks):
                t0 = gc * TG
                nt_ = min(TG, T - 1 - t0)
                s1 = gwork.tile([NT, TG * Bl], F32, tag="s1")
                nc.sync.dma_start(
                    out=s1[:, 0:nt_ * Bl],
                    in_=ins["sel1"][:, t0 * Bl:(t0 + nt_) * Bl])
                s2 = gwork.tile([NT, TG * Bl], F32, tag="s2")
                nc.sync.dma_start(
                    out=s2[:, 0:nt_ * Bl],
                    in_=ins["sel2m"][:, t0 * Bl:(t0 + nt_) * Bl])
                trp = psC.tile([NT, TG * Bl], F32, tag="trp")
                nc.tensor.matmul(trp[:, 0:nt_ * Bl], lhsT=trans_sb[:],
                                 rhs=s1[:, 0:nt_ * Bl], start=True, stop=True)
                mm2 = gwork.tile([NT, TG * Bl], F32, tag="mm2")
                nc.vector.tensor_mul(mm2[:, 0:nt_ * Bl], s2[:, 0:nt_ * Bl],
                                     trp[:, 0:nt_ * Bl])
                mv2 = mm2[:, 0:nt_ * Bl].rearrange("p (t b) -> p b t", b=Bl)
                red2 = gwork.tile([NT, Bl], F32, tag="red2")
                nc.vector.tensor_reduce(red2[:], mv2, axis=AX.X, op=OP.add)
                nc.vector.tensor_add(accT[:], accT[:], red2[:])

            nc.vector.tensor_add(accE[:], accE[:], accT[:])
            gsum = psC.tile([1, Bl], F32, tag="tiny")
            nc.tensor.matmul(gsum[:], lhsT=ones24[:], rhs=accE[:],
                             start=True, stop=False)
            nc.tensor.matmul(gsum[:], lhsT=startv[:], rhs=selstart[:],
                             start=False, stop=False)
            nc.tensor.matmul(gsum[:], lhsT=endv[:], rhs=selend[:],
                             start=False, stop=True)

            # r1 = gold - A   (loss = logZ' - r1)
            r1 = cwork.tile([1, Bl], F32, tag="r1")
            nc.vector.tensor_tensor(r1[:], gsum[:], A_t[:], op=OP.subtract)
            r1p = psC.tile([Bl, 1], F32, tag="tiny")
            nc.tensor.transpose(r1p[:], r1[:], ident[0:1, 0:1])
            loss = cwork.tile([Bl, 1], F32, tag="loss")
            nc.vector.tensor_tensor(loss[:], logZp[:], r1p[:], op=OP.subtract)
            nc.sync.dma_start(out=outs["loss"][:].unsqueeze(1), in_=loss[:])


# ======================= host-side preparation =======================

def make_core_inputs(cfg, x, tags, mask, emb, Wih_f, Whh_f, bih_f, bhh_f,
                     Wih_b, Whh_b, bih_b, bhh_b, W_out, b_out,
                     transitions, start_trans, end_trans):
    """Build the per-core input map (numpy). x/tags/mask are the LOCAL slices
    [Bl, T]."""
    T, Bl, NT, Hd, UT = cfg["T"], cfg["Bl"], cfg["NT"], cfg["Hd"], cfg["UT"]
    R = T * Bl
    M = R // 128
    perm = [0, 1, 3, 2]  # torch gate order (i,f,g,o) -> our (i,f,o,g)

    def reorder_rows(w):  # [4Hd, ...] gate blocks
        blocks = [w[k * Hd:(k + 1) * Hd] for k in perm]
        return np.concatenate(blocks, axis=0)

    def pack_w(wf, wb):  # [4Hd, 128] each -> [128, 2, 4Hd] bf16
        out = np.empty((128, 2, 4 * Hd), dtype=ml_dtypes.bfloat16)
        out[:, 0, :] = reorder_rows(np.asarray(wf, np.float32)).T
        out[:, 1, :] = reorder_rows(np.asarray(wb, np.float32)).T
        return out

    def pack_bias(b):  # [4Hd] -> [128, 4]
        return reorder_rows(np.asarray(b, np.float32)).reshape(4, Hd).T

    biasin = np.empty((128, 2, 2, 4), np.float32)
    biasin[:, 0, 0, :] = pack_bias(bih_f)
    biasin[:, 0, 1, :] = pack_bias(bhh_f)
    biasin[:, 1, 0, :] = pack_bias(bih_b)
    biasin[:, 1, 1, :] = pack_bias(bhh_b)

    W_out = np.asarray(W_out, np.float32)
    wout = np.empty((128, 2, NT), dtype=ml_dtypes.bfloat16)
    wout[:, 0, :] = W_out[:, :Hd].T
    wout[:, 1, :] = W_out[:, Hd:].T

    x = np.asarray(x)
    tags = np.asarray(tags)
    maskf = np.asarray(mask).astype(np.float32)

    # t-major row index: row r = t*Bl + b  ->  x[b, t]
    x_tm = x.T.reshape(-1).astype(np.int32)          # [R]
    idx = x_tm.reshape(M, 128).T.copy()              # [128, M]

    eye = np.eye(NT, dtype=np.float32)
    # w1hot[j, t*Bl+b] = onehot(tags[b,t])[j] * (1 if t==0 else mask[b,t])
    w = maskf.copy()
    w[:, 0] = 1.0
    w1 = eye[tags]                                   # [Bl, T, NT]
    w1 = (w1 * w[:, :, None]).transpose(2, 1, 0)     # [NT, T, Bl]
    w1hot = np.ascontiguousarray(w1.reshape(NT, R), np.float32)

    sel1 = eye[tags[:, :-1]].transpose(2, 1, 0).reshape(NT, (T - 1) * Bl)
    sel1 = np.ascontiguousarray(sel1, np.float32)
    sel2 = eye[tags[:, 1:]] * maskf[:, 1:, None]
    sel2m = np.ascontiguousarray(
        sel2.transpose(2, 1, 0).reshape(NT, (T - 1) * Bl), np.float32)

    selstart = np.ascontiguousarray(eye[tags[:, 0]].T, np.float32)  # [NT, Bl]
    last_idx = np.asarray(mask).sum(axis=1).astype(np.int64) - 1
    last_tags = tags[np.arange(Bl), last_idx]
    selend = np.ascontiguousarray(eye[last_tags].T, np.float32)

    shifts = shift_steps(cfg)
    maskA = maskf[:, shifts].T.reshape(1, -1).astype(np.float32)  # [1,nsh*Bl]
    maskc = np.broadcast_to(maskf[:, UT:].T.reshape(1, -1),
                            (NT, (T - UT) * Bl))
    maskc = np.ascontiguousarray(maskc, np.uint8)

    return {
        "emb": np.ascontiguousarray(emb, np.float32),
        "idx": idx,
        "wih": pack_w(Wih_f, Wih_b),
        "whh": pack_w(Whh_f, Whh_b),
        "wout": wout,
        "biasin": biasin,
        "bout": np.asarray(b_out, np.float32).reshape(NT, 1),
        "trans": np.ascontiguousarray(transitions, np.float32),
        "startv": np.asarray(start_trans, np.float32).reshape(NT, 1),
        "endv": np.asarray(end_trans, np.float32).reshape(NT, 1),
        "selstart": selstart,
        "selend": selend,
        "maskA": maskA,
        "maskc": maskc,
        "w1hot": w1hot,
        "sel1": sel1,
        "sel2m": sel2m,
    }


def input_specs(cfg):
    T, Bl, NT, Hd, UT, V = (cfg["T"], cfg["Bl"], cfg["NT"], cfg["Hd"],
                            cfg["UT"], cfg["V"])
    R = T * Bl
    M = R // 128
    nsh = len(shift_steps(cfg))
    return {
        "emb": ([V, 128], F32),
        "idx": ([128, M], I32),
        "wih": ([128, 2, 4 * Hd], BF16),
        "whh": ([128, 2, 4 * Hd], BF16),
        "wout": ([128, 2, NT], BF16),
        "biasin": ([128, 2, 2, 4], F32),
        "bout": ([NT, 1], F32),
        "trans": ([NT, NT], F32),
        "startv": ([NT, 1], F32),
        "endv": ([NT, 1], F32),
        "selstart": ([NT, Bl], F32),
        "selend": ([NT, Bl], F32),
        "maskA": ([1, nsh * Bl], F32),
        "maskc": ([NT, (T - UT) * Bl], U8),
        "w1hot": ([NT, R], F32),
        "sel1": ([NT, (T - 1) * Bl], F32),
        "sel2m": ([NT, (T - 1) * Bl], F32),
    }


_BUILT = {}


def build_program(cfg, num_devices=8):
    key = tuple(sorted(cfg.items()))
    if key in _BUILT:
        return _BUILT[key]
    nc = bacc.Bacc("TRN2", target_bir_lowering=False, debug=False,
                   num_devices=num_devices)
    ins = {}
    for name, (shape, dt_) in input_specs(cfg).items():
        ins[name] = nc.dram_tensor(name, shape, dt_, kind="ExternalInput").ap()
    outs = {"loss": nc.dram_tensor("loss", [cfg["Bl"]], F32,
                                   kind="ExternalOutput").ap()}
    with tile.TileContext(nc) as tc:
        build_body(tc, outs, ins, cfg)
    nc.compile()
    _BUILT[key] = nc
    return nc


def kernel(**inputs):
    from concourse.bass_utils import run_bass_kernel_spmd

    cfg = full_cfg()
    Bl = cfg["Bl"]
    B = 128
    n_cores = B // Bl
    nc = build_program(cfg, num_devices=n_cores)

    np_in = {k: np.asarray(v) for k, v in inputs.items()}
    in_maps = []
    for c in range(n_cores):
        sl = slice(c * Bl, (c + 1) * Bl)
        in_maps.append(make_core_inputs(
            cfg,
            np_in["x"][sl], np_in["tags"][sl], np_in["mask"][sl],
            np_in["emb"],
            np_in["Wih_f"], np_in["Whh_f"], np_in["bih_f"], np_in["bhh_f"],
            np_in["Wih_b"], np_in["Whh_b"], np_in["bih_b"], np_in["bhh_b"],
            np_in["W_out"], np_in["b_out"], np_in["transitions"],
            np_in["start_trans"], np_in["end_trans"]))

    res = run_bass_kernel_spmd(nc, in_maps, core_ids=list(range(n_cores)),
                               trace=TRACE)
    if res.exec_time_ns is not None:
        LAST_EXEC_NS.append(res.exec_time_ns)
    vals = np.concatenate([res.results[c]["loss"] for c in range(n_cores)])
    return np.float32(vals.mean())


TRACE = False
LAST_EXEC_NS = []


# revision 33
# speedup vs baseline: 54.2750x; 54.2750x over previous
"""BiLSTM-CRF loss kernel for Trainium2 (8 NeuronCores, data-parallel over batch).

Self-contained: hardcodes shapes B=128, T=512, V=50000, NT=24, E=128, H=256.
Each core processes 16 examples end-to-end (embedding gather, BiLSTM,
emissions, CRF forward logZ, gold path score); host only reorders inputs into
DMA-friendly layouts, builds one-hot index selectors, and averages the 128
per-example (logZ - gold) values.
"""

import sys

for _p in ("/opt/trn_rl_repo",):
    if _p not in sys.path:
        sys.path.insert(0, _p)

import numpy as np
import ml_dtypes

import concourse.bass as bass
import concourse.bacc as bacc
import concourse.tile as tile
from concourse import mybir
from concourse.bass import IndirectOffsetOnAxis
from concourse.masks import make_identity

F32 = mybir.dt.float32
BF16 = mybir.dt.bfloat16
I32 = mybir.dt.int32
U8 = mybir.dt.uint8
AX = mybir.AxisListType
OP = mybir.AluOpType
ACTF = mybir.ActivationFunctionType


def full_cfg():
    return dict(T=512, Bl=16, V=50000, NT=24, E=128, Hd=128, Tc=32, UT=256,
                EC=512, TG=32)


def shift_steps(cfg):
    # steps at which the CRF running score is re-based (every 4th step keeps
    # |log q| < ~60, safely inside f32 exp range)
    return [t for t in range(4, cfg["T"], 4)]


def build_body(tc, outs, ins, cfg):
    """Emit the whole per-core program inside an open TileContext.

    outs/ins: dicts name -> bass.AP (DRAM).
    """
    nc = tc.nc
    T, Bl, NT, Hd = cfg["T"], cfg["Bl"], cfg["NT"], cfg["Hd"]
    Tc, UT, EC, TG = cfg["Tc"], cfg["UT"], cfg["EC"], cfg["TG"]
    R = T * Bl                  # total (t, b) rows
    M = R // 128                # 128-row tiles
    NCH = T // Tc               # lstm chunks
    RTC = Tc * Bl // 128        # row-tiles per chunk
    G4 = 4 * Hd
    shifts = shift_steps(cfg)
    shift_of = {t: i for i, t in enumerate(shifts)}

    import contextlib
    ctx = contextlib.ExitStack()
    with ctx:
        const = ctx.enter_context(tc.tile_pool(name="const", bufs=1))
        big = ctx.enter_context(tc.tile_pool(name="big", bufs=1))
        work = ctx.enter_context(tc.tile_pool(name="work", bufs=3))

        # ---------------- constants ----------------
        ident = const.tile([128, 128], F32)
        make_identity(nc, ident[:])

        idx_sb = const.tile([128, M], I32)
        nc.sync.dma_start(out=idx_sb[:], in_=ins["idx"][:])

        wih_sb = const.tile([128, 2, G4], BF16)
        nc.sync.dma_start(out=wih_sb[:], in_=ins["wih"][:])
        whh_sb = const.tile([128, 2, G4], BF16)
        nc.sync.dma_start(out=whh_sb[:], in_=ins["whh"][:])
        wout_sb = const.tile([128, 2, NT], BF16)
        nc.sync.dma_start(out=wout_sb[:], in_=ins["wout"][:])

        biasin = const.tile([128, 2, 2, 4], F32)
        nc.sync.dma_start(out=biasin[:], in_=ins["biasin"][:])
        bias_sb = const.tile([128, 2, 4], F32)
        nc.vector.tensor_add(bias_sb[:], biasin[:, :, 0, :], biasin[:, :, 1, :])

        trans_sb = const.tile([NT, NT], F32)
        nc.sync.dma_start(out=trans_sb[:], in_=ins["trans"][:])
        E_sb = const.tile([NT, NT], F32)
        nc.scalar.activation(E_sb[:], trans_sb[:], ACTF.Exp)

        bout_sb = const.tile([NT, 1], F32)
        nc.sync.dma_start(out=bout_sb[:], in_=ins["bout"][:])
        startv = const.tile([NT, 1], F32)
        nc.sync.dma_start(out=startv[:], in_=ins["startv"][:])
        endv = const.tile([NT, 1], F32)
        nc.sync.dma_start(out=endv[:], in_=ins["endv"][:])
        selstart = const.tile([NT, Bl], F32)
        nc.sync.dma_start(out=selstart[:], in_=ins["selstart"][:])
        selend = const.tile([NT, Bl], F32)
        nc.sync.dma_start(out=selend[:], in_=ins["selend"][:])

        nsh = len(shifts)
        maskA = const.tile([1, nsh * Bl], F32)
        nc.sync.dma_start(out=maskA[:], in_=ins["maskA"][:])
        EV = cfg["EV"]
        snap_sb = const.tile([NT, max(len(EV), 1)], U8)
        nc.sync.dma_start(out=snap_sb[:], in_=ins["snapmask"][:])

        ones24 = const.tile([NT, 1], F32)
        nc.vector.memset(ones24[:], 1.0)
        ones1 = const.tile([1, NT], F32)
        nc.vector.memset(ones1[:], 1.0)

        # ---------------- phase A: gather + transpose embeddings ----------------
        xeT = big.tile([128, R], BF16)
        with tc.tile_pool(name="psA", bufs=1, space="PSUM") as psA, \
             tc.tile_pool(name="psG", bufs=3, space="PSUM") as psG, \
             tc.tile_pool(name="gath", bufs=3) as gath:
            gather_order = []
            for i in range((M + 1) // 2):
                gather_order.append(M - 1 - i)
                if i != M - 1 - i:
                    gather_order.append(i)
            for m in gather_order:
                xe_raw = gath.tile([128, 128], F32, tag="xe_raw")
                nc.gpsimd.indirect_dma_start(
                    out=xe_raw[:], out_offset=None,
                    in_=ins["emb"][:],
                    in_offset=IndirectOffsetOnAxis(ap=idx_sb[:, m:m + 1], axis=0),
                )
                xe_ps = psA.tile([128, 128], F32, tag="xe_ps")
                nc.tensor.transpose(xe_ps[:], xe_raw[:], ident[:])
                nc.vector.tensor_copy(xeT[:, m * 128:(m + 1) * 128], xe_ps[:])

            # ---------------- phase B: BiLSTM ----------------
            h_f = big.tile([128, R], BF16)
            h_b = big.tile([128, R], BF16)
            zero_h = const.tile([128, 2 * Bl], BF16)
            nc.vector.memset(zero_h[:], 0.0)
            c_f = big.tile([128, Bl], F32)
            nc.vector.memset(c_f[:], 0.0)
            c_b = big.tile([128, Bl], F32)
            nc.vector.memset(c_b[:], 0.0)

            # bias as a K=4 matmul: biasT [4, dir, 128] (transposed bias) and
            # a one-hot selector so one accumulating matmul adds bias[j,k] to
            # every (k, b) column of the gate PSUM tile.
            biasT = const.tile([4, 2, 128], BF16)
            sel4 = const.tile([4, 4 * Bl], BF16)
            nc.sync.dma_start(out=sel4[:], in_=ins["sel4"][:])
            for d in range(2):
                bt_ps = psA.tile([4, 128], F32, tag="bt_ps")
                nc.tensor.transpose(bt_ps[:], bias_sb[:, d, :], ident[:])
                nc.vector.tensor_copy(biasT[:, d, :], bt_ps[:])

            with tc.tile_pool(name="lwork", bufs=3) as lwork:
                # zero_b: bwd's initial h, made to *depend on* fwd's first
                # sigmoid so the bwd chain starts half a round later and the
                # two chains stay phase-offset (latencies are identical, so
                # the initial offset persists).
                zero_b = const.tile([128, Bl], BF16)
                pend_bwd = None
                if True:
                    def lstm_h1(d, t):
                        # first half: all of g accumulated in PSUM by the PE
                        # (recurrence + input projection + bias), then sigmoid
                        # straight off PSUM.
                        h_st = h_f if d == 0 else h_b
                        rhs = (h_st[:, (t - 1) * Bl:t * Bl] if d == 0 and t > 0
                               else h_st[:, (t + 1) * Bl:(t + 2) * Bl]
                               if d == 1 and t < T - 1 else
                               (zero_h[:, 0:Bl] if d == 0 else zero_b[:]))
                        gps = psG.tile([128, 4, Bl], F32, tag=f"gps{d}")
                        for k in range(4):
                            nc.tensor.matmul(
                                gps[:, k, :],
                                lhsT=whh_sb[:, d, k * 128:(k + 1) * 128],
                                rhs=rhs, start=(k == 0), stop=False)
                        for k in range(4):
                            nc.tensor.matmul(
                                gps[:, k, :],
                                lhsT=wih_sb[:, d, k * 128:(k + 1) * 128],
                                rhs=xeT[:, t * Bl:(t + 1) * Bl],
                                start=False, stop=False)
                        nc.tensor.matmul(
                            gps[:].rearrange("p k b -> p (k b)"),
                            lhsT=biasT[:, d, :], rhs=sel4[:],
                            start=False, stop=True)
                        # all-sigmoid cell: weights pre-scaled on host so
                        # tanh(x) = 2*sig(2x)-1 and h is stored as h/2.
                        sg = lwork.tile([128, 4, Bl], F32, tag=f"sg{d}")
                        nc.scalar.activation(sg[:], gps[:], ACTF.Sigmoid)
                        return sg

                    def lstm_h2(d, t, sg, c_st):
                        # second half: cell update + h output
                        h_st = h_f if d == 0 else h_b
                        m1 = lwork.tile([128, Bl], F32, tag=f"m1{d}")
                        nc.vector.scalar_tensor_tensor(
                            m1[:], sg[:, 3, :], 0.5, sg[:, 0, :],
                            op0=OP.subtract, op1=OP.mult)
                        m2 = lwork.tile([128, Bl], F32, tag=f"m2{d}")
                        nc.gpsimd.tensor_mul(m2[:], sg[:, 1, :], c_st)
                        nc.vector.scalar_tensor_tensor(
                            c_st, m1[:], 2.0, m2[:],
                            op0=OP.mult, op1=OP.add)
                        tcn = lwork.tile([128, Bl], F32, tag=f"tcn{d}")
                        nc.scalar.activation(tcn[:], c_st, ACTF.Sigmoid,
                                             scale=2.0)
                        nc.vector.scalar_tensor_tensor(
                            h_st[:, t * Bl:(t + 1) * Bl],
                            tcn[:], 0.5, sg[:, 2, :],
                            op0=OP.subtract, op1=OP.mult)

                    # software-pipelined emission: bwd runs half a step behind
                    # fwd so the two chains' engine visits interleave.
                    for s in range(T):
                        sgf = lstm_h1(0, s)
                        if s == 0:
                            nc.vector.tensor_scalar_mul(zero_b[:],
                                                        sgf[:, 0, :], 0.0)
                        if pend_bwd is not None:
                            lstm_h2(1, pend_bwd[0], pend_bwd[1], c_b[:])
                        sgb = lstm_h1(1, T - 1 - s)
                        lstm_h2(0, s, sgf, c_f[:])
                        pend_bwd = (T - 1 - s, sgb)
                lstm_h2(1, pend_bwd[0], pend_bwd[1], c_b[:])

        # ---------------- phase C: emissions ----------------
        emT = big.tile([NT, R], F32)
        EM = big.tile([NT, R], F32)
        with tc.tile_pool(name="psB", bufs=2, space="PSUM") as psB:
            n_ec = R // EC
            ec_t = EC // Bl
            ec_order = sorted(range(n_ec),
                              key=lambda c: max((c + 1) * ec_t,
                                                T - 1 - c * ec_t))
            for ec in ec_order:
                ps = psB.tile([NT, EC], F32, tag="em_ps")
                nc.tensor.matmul(ps[:], lhsT=wout_sb[:, 0, :],
                                 rhs=h_f[:, ec * EC:(ec + 1) * EC],
                                 start=True, stop=False)
                nc.tensor.matmul(ps[:], lhsT=wout_sb[:, 1, :],
                                 rhs=h_b[:, ec * EC:(ec + 1) * EC],
                                 start=False, stop=True)
                nc.vector.tensor_scalar_add(emT[:, ec * EC:(ec + 1) * EC],
                                            ps[:], bout_sb[:])
                nc.scalar.activation(EM[:, ec * EC:(ec + 1) * EC],
                                     emT[:, ec * EC:(ec + 1) * EC], ACTF.Exp)

        # ---------------- phase D: CRF forward + gold ----------------
        # exp-domain CRF: q_t = exp(score'_t); per step q <- (E^T q) * exp(em_t)
        # with periodic rebase q <- q / q[0] (offsets accumulated via logs at
        # the end). No activation-table switches inside the loop. EM is
        # exponentiated per emission chunk so the CRF overlaps emissions.
        with tc.tile_pool(name="psC", bufs=2, space="PSUM") as psC, \
             tc.tile_pool(name="psD", bufs=1, space="PSUM") as psD, \
             tc.tile_pool(name="cwork", bufs=3) as cwork, \
             tc.tile_pool(name="gwork", bufs=2) as gwork:
            estart = cwork.tile([NT, 1], F32, tag="estart")
            nc.scalar.activation(estart[:], startv[:], ACTF.Exp)
            NHALF = 1
            Hb = Bl // NHALF
            q_half = []
            q0_hist = big.tile([1, max(nsh, 1) * Bl], F32)
            for hh in range(NHALF):
                qh = big.tile([NT, Hb], F32)
                nc.vector.tensor_scalar_mul(
                    qh[:], EM[:, hh * Hb:(hh + 1) * Hb], estart[:])
                q_half.append(qh)

            def crf_step(hh, t):
                q = q_half[hh]
                o = hh * Hb
                if t in shift_of:
                    si = shift_of[t]
                    nc.vector.tensor_copy(
                        q0_hist[0:1, si * Bl + o:si * Bl + o + Hb], q[0:1, :])
                    rc = cwork.tile([1, Hb], F32, tag=f"rc{hh}")
                    nc.vector.reciprocal(rc[:], q[0:1, :])
                    ob = psD.tile([NT, Hb], F32, tag=f"ob{hh}")
                    nc.tensor.matmul(ob[:], lhsT=ones1[:], rhs=rc[:],
                                     start=True, stop=True)
                    qs = cwork.tile([NT, Hb], F32, tag=f"qs{hh}")
                    nc.vector.tensor_mul(qs[:], q[:], ob[:])
                    rhs_mm = qs
                else:
                    rhs_mm = q
                Pp = psC.tile([NT, Hb], F32, tag=f"Pp{hh}")
                nc.tensor.matmul(Pp[:], lhsT=E_sb[:], rhs=rhs_mm[:],
                                 start=True, stop=True)
                em_sl = EM[:, t * Bl + o:t * Bl + o + Hb]
                nc.vector.tensor_mul(q[:], Pp[:], em_sl)

            # per-example final-score snapshots: event e freezes column b at
            # step t; the per-core snapmask column enables only this core's
            # own freeze events (the event list is the union over cores).
            qfinal = big.tile([NT, Bl], F32)
            ev_at = {}
            for e, (t_, b_) in enumerate(EV):
                ev_at.setdefault(t_, []).append((e, b_))
            for t in range(1, T):
                for hh in range(NHALF):
                    crf_step(hh, t)
                for e, b_ in ev_at.get(t, []):
                    nc.vector.copy_predicated(
                        qfinal[:, b_:b_ + 1], snap_sb[:, e:e + 1],
                        q_half[b_ // Hb][:, b_ % Hb:b_ % Hb + 1])

            # logZ' = LSE_i(ln q_i + end_i) per example
            scoreT = cwork.tile([NT, Bl], F32, tag="scoreT")
            nc.scalar.activation(scoreT[:], qfinal[:], ACTF.Ln)
            sce = cwork.tile([NT, Bl], F32, tag="sce")
            nc.vector.tensor_scalar_add(sce[:], scoreT[:], endv[:])
            tp = psD.tile([Bl, NT], F32, tag="tiny")
            nc.tensor.transpose(tp[:], sce[:], ident[0:NT, 0:NT])
            sc2 = cwork.tile([Bl, NT], F32, tag="sc2")
            nc.vector.tensor_copy(sc2[:], tp[:])
            nmx = cwork.tile([Bl, 1], F32, tag="nmx")
            nc.vector.tensor_reduce(nmx[:], sc2[:], axis=AX.X, op=OP.max,
                                    negate=True)
            pe2 = cwork.tile([Bl, NT], F32, tag="pe2")
            se = cwork.tile([Bl, 1], F32, tag="se")
            nc.scalar.activation(pe2[:], sc2[:], ACTF.Exp, bias=nmx[:],
                                 accum_out=se[:])
            lse = cwork.tile([Bl, 1], F32, tag="lse")
            nc.scalar.activation(lse[:], se[:], ACTF.Ln)
            logZp = cwork.tile([Bl, 1], F32, tag="logZp")
            nc.vector.tensor_tensor(logZp[:], lse[:], nmx[:], op=OP.subtract)

            # A_b = sum over shift steps of ln(q0) * mask
            lnq0 = big.tile([1, max(nsh, 1) * Bl], F32)
            nc.scalar.activation(lnq0[:], q0_hist[:], ACTF.Ln)
            ohm = big.tile([1, max(nsh, 1) * Bl], F32)
            nc.vector.tensor_mul(ohm[:], lnq0[:], maskA[:])
            A_t = cwork.tile([1, Bl], F32, tag="A_t")
            ohm_v = ohm[:].rearrange("p (s b) -> p b s", b=Bl)
            nc.vector.tensor_reduce(A_t[:], ohm_v, axis=AX.X, op=OP.add)

            # ---- gold path score ----
            accE = gwork.tile([NT, Bl], F32, tag="accE")
            nc.vector.memset(accE[:], 0.0)
            accT = gwork.tile([NT, Bl], F32, tag="accT")
            nc.vector.memset(accT[:], 0.0)

            # emission term: sum_t w1hot[:, t, b] * emT[:, t, b]
            n_em_chunks = (T + TG - 1) // TG
            for gc in range(n_em_chunks):
                t0 = gc * TG
                nt_ = min(TG, T - t0)
                w1 = gwork.tile([NT, TG * Bl], F32, tag="w1")
                nc.sync.dma_start(
                    out=w1[:, 0:nt_ * Bl],
                    in_=ins["w1hot"][:, t0 * Bl:(t0 + nt_) * Bl])
                mm1 = gwork.tile([NT, TG * Bl], F32, tag="mm1")
                nc.vector.tensor_mul(mm1[:, 0:nt_ * Bl], w1[:, 0:nt_ * Bl],
                                     emT[:, t0 * Bl:(t0 + nt_) * Bl])
                mv = mm1[:, 0:nt_ * Bl].rearrange("p (t b) -> p b t", b=Bl)
                red = gwork.tile([NT, Bl], F32, tag="red")
                nc.vector.tensor_reduce(red[:], mv, axis=AX.X, op=OP.add)
                nc.vector.tensor_add(accE[:], accE[:], red[:])

            # transition term: sum_t sel2m[:,t,b] * (trans^T @ sel1)[:,t,b]
            n_tr_chunks = (T - 1 + TG - 1) // TG
            for gc in range(n_tr_chunks):
                t0 = gc * TG
                nt_ = min(TG, T - 1 - t0)
                s1 = gwork.tile([NT, TG * Bl], F32, tag="s1")
                nc.sync.dma_start(
                    out=s1[:, 0:nt_ * Bl],
                    in_=ins["sel1"][:, t0 * Bl:(t0 + nt_) * Bl])
                s2 = gwork.tile([NT, TG * Bl], F32, tag="s2")
                nc.sync.dma_start(
                    out=s2[:, 0:nt_ * Bl],
                    in_=ins["sel2m"][:, t0 * Bl:(t0 + nt_) * Bl])
                trp = psD.tile([NT, TG * Bl], F32, tag="trp")
                nc.tensor.matmul(trp[:, 0:nt_ * Bl], lhsT=trans_sb[:],
                                 rhs=s1[:, 0:nt_ * Bl], start=True, stop=True)
                mm2 = gwork.tile([NT, TG * Bl], F32, tag="mm2")
                nc.vector.tensor_mul(mm2[:, 0:nt_ * Bl], s2[:, 0:nt_ * Bl],
                                     trp[:, 0:nt_ * Bl])
                mv2 = mm2[:, 0:nt_ * Bl].rearrange("p (t b) -> p b t", b=Bl)
                red2 = gwork.tile([NT, Bl], F32, tag="red2")
                nc.vector.tensor_reduce(red2[:], mv2, axis=AX.X, op=OP.add)
                nc.vector.tensor_add(accT[:], accT[:], red2[:])

            nc.vector.tensor_add(accE[:], accE[:], accT[:])
            gsum = psD.tile([1, Bl], F32, tag="tiny")
            nc.tensor.matmul(gsum[:], lhsT=ones24[:], rhs=accE[:],
                             start=True, stop=False)
            nc.tensor.matmul(gsum[:], lhsT=startv[:], rhs=selstart[:],
                             start=False, stop=False)
            nc.tensor.matmul(gsum[:], lhsT=endv[:], rhs=selend[:],
                             start=False, stop=True)

            # r1 = gold - A   (loss = logZ' - r1)
            r1 = cwork.tile([1, Bl], F32, tag="r1")
            nc.vector.tensor_tensor(r1[:], gsum[:], A_t[:], op=OP.subtract)
            r1p = psD.tile([Bl, 1], F32, tag="tiny")
            nc.tensor.transpose(r1p[:], r1[:], ident[0:1, 0:1])
            loss = cwork.tile([Bl, 1], F32, tag="loss")
            nc.vector.tensor_tensor(loss[:], logZp[:], r1p[:], op=OP.subtract)
            nc.sync.dma_start(out=outs["loss"][:].unsqueeze(1), in_=loss[:])


# ======================= host-side preparation =======================

def make_core_inputs(cfg, x, tags, mask, emb, Wih_f, Whh_f, bih_f, bhh_f,
                     Wih_b, Whh_b, bih_b, bhh_b, W_out, b_out,
                     transitions, start_trans, end_trans):
    """Build the per-core input map (numpy). x/tags/mask are the LOCAL slices
    [Bl, T]."""
    T, Bl, NT, Hd, UT = cfg["T"], cfg["Bl"], cfg["NT"], cfg["Hd"], cfg["UT"]
    R = T * Bl
    M = R // 128
    perm = [0, 1, 3, 2]  # torch gate order (i,f,g,o) -> our (i,f,o,g)

    # Gate scale factors fold the all-sigmoid cell rewrite into the weights:
    # tanh(x)=2*sig(2x)-1 needs gate-g pre-activations doubled; h is stored
    # as h/2 so everything consuming h (Whh, W_out) is doubled.
    WIH_S = np.array([1.0, 1.0, 1.0, 2.0], np.float32)   # (i,f,o,g)
    WHH_S = np.array([2.0, 2.0, 2.0, 4.0], np.float32)
    BIA_S = WIH_S

    def reorder_rows(w, scales):  # [4Hd, ...] gate blocks
        blocks = [w[k * Hd:(k + 1) * Hd] * s for k, s in zip(perm, scales)]
        return np.concatenate(blocks, axis=0)

    def pack_w(wf, wb, scales):  # [4Hd, 128] each -> [128, 2, 4Hd] bf16
        out = np.empty((128, 2, 4 * Hd), dtype=ml_dtypes.bfloat16)
        out[:, 0, :] = reorder_rows(np.asarray(wf, np.float32), scales).T
        out[:, 1, :] = reorder_rows(np.asarray(wb, np.float32), scales).T
        return out

    def pack_bias(b):  # [4Hd] -> [128, 4]
        return reorder_rows(np.asarray(b, np.float32), BIA_S).reshape(4, Hd).T

    biasin = np.empty((128, 2, 2, 4), np.float32)
    biasin[:, 0, 0, :] = pack_bias(bih_f)
    biasin[:, 0, 1, :] = pack_bias(bhh_f)
    biasin[:, 1, 0, :] = pack_bias(bih_b)
    biasin[:, 1, 1, :] = pack_bias(bhh_b)

    W_out = np.asarray(W_out, np.float32) * 2.0   # h stored as h/2
    wout = np.empty((128, 2, NT), dtype=ml_dtypes.bfloat16)
    wout[:, 0, :] = W_out[:, :Hd].T
    wout[:, 1, :] = W_out[:, Hd:].T

    x = np.asarray(x)
    tags = np.asarray(tags)
    maskf = np.asarray(mask).astype(np.float32)

    # t-major row index: row r = t*Bl + b  ->  x[b, t]
    x_tm = x.T.reshape(-1).astype(np.int32)          # [R]
    idx = x_tm.reshape(M, 128).T.copy()              # [128, M]

    eye = np.eye(NT, dtype=np.float32)
    # w1hot[j, t*Bl+b] = onehot(tags[b,t])[j] * (1 if t==0 else mask[b,t])
    w = maskf.copy()
    w[:, 0] = 1.0
    w1 = eye[tags]                                   # [Bl, T, NT]
    w1 = (w1 * w[:, :, None]).transpose(2, 1, 0)     # [NT, T, Bl]
    w1hot = np.ascontiguousarray(w1.reshape(NT, R), np.float32)

    sel1 = eye[tags[:, :-1]].transpose(2, 1, 0).reshape(NT, (T - 1) * Bl)
    sel1 = np.ascontiguousarray(sel1, np.float32)
    sel2 = eye[tags[:, 1:]] * maskf[:, 1:, None]
    sel2m = np.ascontiguousarray(
        sel2.transpose(2, 1, 0).reshape(NT, (T - 1) * Bl), np.float32)

    selstart = np.ascontiguousarray(eye[tags[:, 0]].T, np.float32)  # [NT, Bl]
    last_idx = np.asarray(mask).sum(axis=1).astype(np.int64) - 1
    last_tags = tags[np.arange(Bl), last_idx]
    selend = np.ascontiguousarray(eye[last_tags].T, np.float32)

    shifts = shift_steps(cfg)
    maskA = maskf[:, shifts].T.reshape(1, -1).astype(np.float32)  # [1,nsh*Bl]
    EV = cfg["EV"]
    fz = np.asarray(mask).sum(axis=1).astype(np.int64) - 1   # freeze step per b
    snapmask = np.zeros((NT, max(len(EV), 1)), np.uint8)
    for e, (t_, b_) in enumerate(EV):
        if fz[b_] == t_:
            snapmask[:, e] = 1

    return {
        "emb": np.ascontiguousarray(emb, np.float32),
        "idx": idx,
        "wih": pack_w(Wih_f, Wih_b, WIH_S),
        "whh": pack_w(Whh_f, Whh_b, WHH_S),
        "wout": wout,
        "biasin": biasin,
        "bout": np.asarray(b_out, np.float32).reshape(NT, 1),
        "trans": np.ascontiguousarray(transitions, np.float32),
        "startv": np.asarray(start_trans, np.float32).reshape(NT, 1),
        "endv": np.asarray(end_trans, np.float32).reshape(NT, 1),
        "selstart": selstart,
        "selend": selend,
        "maskA": maskA,
        "snapmask": snapmask,
        "w1hot": w1hot,
        "sel1": sel1,
        "sel4": np.kron(np.eye(4, dtype=np.float32),
                        np.ones((1, Bl), np.float32)).astype(ml_dtypes.bfloat16),
        "sel2m": sel2m,
    }


def input_specs(cfg):
    T, Bl, NT, Hd, UT, V = (cfg["T"], cfg["Bl"], cfg["NT"], cfg["Hd"],
                            cfg["UT"], cfg["V"])
    R = T * Bl
    M = R // 128
    nsh = len(shift_steps(cfg))
    return {
        "emb": ([V, 128], F32),
        "idx": ([128, M], I32),
        "wih": ([128, 2, 4 * Hd], BF16),
        "whh": ([128, 2, 4 * Hd], BF16),
        "wout": ([128, 2, NT], BF16),
        "biasin": ([128, 2, 2, 4], F32),
        "bout": ([NT, 1], F32),
        "trans": ([NT, NT], F32),
        "startv": ([NT, 1], F32),
        "endv": ([NT, 1], F32),
        "selstart": ([NT, Bl], F32),
        "selend": ([NT, Bl], F32),
        "maskA": ([1, nsh * Bl], F32),
        "snapmask": ([NT, max(len(cfg["EV"]), 1)], U8),
        "w1hot": ([NT, R], F32),
        "sel1": ([NT, (T - 1) * Bl], F32),
        "sel4": ([4, 4 * cfg["Bl"]], BF16),
        "sel2m": ([NT, (T - 1) * Bl], F32),
    }


_BUILT = {}


def build_program(cfg, num_devices=8):
    key = tuple(sorted(cfg.items()))
    if key in _BUILT:
        return _BUILT[key]
    nc = bacc.Bacc("TRN2", target_bir_lowering=False, debug=False,
                   num_devices=num_devices)
    ins = {}
    for name, (shape, dt_) in input_specs(cfg).items():
        ins[name] = nc.dram_tensor(name, shape, dt_, kind="ExternalInput").ap()
    outs = {"loss": nc.dram_tensor("loss", [cfg["Bl"]], F32,
                                   kind="ExternalOutput").ap()}
    with tile.TileContext(nc) as tc:
        build_body(tc, outs, ins, cfg)
    nc.compile()
    _BUILT[key] = nc
    return nc


def kernel(**inputs):
    from concourse.bass_utils import run_bass_kernel_spmd

    cfg = full_cfg()
    Bl = cfg["Bl"]
    B = 128
    n_cores = B // Bl

    np_in = {k: np.asarray(v) for k, v in inputs.items()}
    # freeze-snapshot events: union over cores of (freeze step, local column)
    fz_all = np_in["mask"].sum(axis=1).astype(np.int64) - 1
    ev = sorted({(int(fz_all[b]), b % Bl) for b in range(B)})
    cfg = dict(cfg, EV=tuple(ev))
    nc = build_program(cfg, num_devices=n_cores)
    in_maps = []
    for c in range(n_cores):
        sl = slice(c * Bl, (c + 1) * Bl)
        in_maps.append(make_core_inputs(
            cfg,
            np_in["x"][sl], np_in["tags"][sl], np_in["mask"][sl],
            np_in["emb"],
            np_in["Wih_f"], np_in["Whh_f"], np_in["bih_f"], np_in["bhh_f"],
            np_in["Wih_b"], np_in["Whh_b"], np_in["bih_b"], np_in["bhh_b"],
            np_in["W_out"], np_in["b_out"], np_in["transitions"],
            np_in["start_trans"], np_in["end_trans"]))

    res = run_bass_kernel_spmd(nc, in_maps, core_ids=list(range(n_cores)),
                               trace=TRACE)
    if res.exec_time_ns is not None:
        LAST_EXEC_NS.append(res.exec_time_ns)
    vals = np.concatenate([res.results[c]["loss"] for c in range(n_cores)])
    return np.float32(vals.mean())


TRACE = False
LAST_EXEC_NS = []
